# revision 10
# baseline (speedup 1.0000x reference)
"""MLA attention (DeepSeek-style, absorbed weights) on 8 TRN2 NeuronCores.

Sharding: heads are tensor-parallel (2 heads/core, output channel blocks are
per-head independent in the absorbed formulation, so no output all-reduce).
The shared latent projections (c_q, c_kv, roped k_r) are token-parallel
(512 tokens/core) and exchanged with a single packed AllGather.

All matmuls run as float32r (TF32-like, 1 cycle/row on the PE at moving
dim >= 256).  Attention is computed in a transposed orientation
logitsT[s, t] so that both the QK^T and AV matmuls need no on-chip
transposes; softmax needs no max subtraction (logits are O(1) for this
problem) and the column sums come from an appended ones-row matmul.
"""

import math

import numpy as np

B, T, C = 4, 1024, 2048
NH, HS = 16, 128
NLQ = NLKV = 512
DHR = 64
NCORES = 8
HPC = NH // NCORES          # heads per core
TOK = (B * T) // NCORES     # tokens per core (half a batch)
SCALE = 1.0 / math.sqrt(HS + DHR)

# packed AllGather shard layout (rows of a [SH_ROWS, TOK] f32 tensor)
ROW_CQ = 0            # c_qT_own   [NLQ, TOK]
ROW_CKVT = NLQ        # c_kvT_own  [NLKV, TOK]
ROW_CKV = 2 * NLQ     # c_kv_own   [TOK, NLKV]  (t-major)
ROW_KR = 3 * NLQ      # k_rT_own   [DHR, TOK]
SH_ROWS = 3 * NLQ + DHR  # 1600

_cache = {}


def _build():
    import concourse.mybir as mybir
    import concourse.tile as tile
    from concourse import bacc

    f32 = mybir.dt.float32
    f32r = mybir.dt.float32r
    Exp = mybir.ActivationFunctionType.Exp
    mult = mybir.AluOpType.mult
    add = mybir.AluOpType.add

    nc = bacc.Bacc(trn_type="TRN2", num_devices=NCORES)
    P = nc.declare_dram_parameter

    xT = P("xT", [C, TOK], f32r, isOutput=False)
    wdqT = P("wdqT", [C, NLQ], f32r, isOutput=False)
    wdkvT = P("wdkvT", [C, NLKV], f32r, isOutput=False)
    wkr2T = P("wkr2T", [C, 2 * DHR], f32r, isOutput=False)
    wuqT = P("wuqT", [HPC * HS, NLQ], f32r, isOutput=False)
    wuk = P("wuk", [HPC * HS, NLKV], f32r, isOutput=False)
    wqr2T = P("wqr2T", [HPC * NLQ, 2 * DHR], f32r, isOutput=False)
    wo2T = P("wo2T", [C, HPC * HS], f32r, isOutput=False)
    wuv = P("wuv", [C, NLKV], f32r, isOutput=False)
    cos2 = P("cos2", [DHR, T], f32r, isOutput=False)
    sin2 = P("sin2", [DHR, T], f32r, isOutput=False)
    cos2o = P("cos2o", [DHR, TOK], f32r, isOutput=False)
    sin2o = P("sin2o", [DHR, TOK], f32r, isOutput=False)
    maskp = P("maskp", [128, 128], f32r, isOutput=False)
    out = P("out", [HPC * HS, B * T], f32, isOutput=True)

    CC = C // 128  # 16 c-chunks

    with tile.TileContext(nc) as tc:
        with (
            tc.tile_pool(name="pres", bufs=1) as pres,
            tc.tile_pool(name="dram", bufs=1, space="DRAM") as dram,
            tc.tile_pool(name="ps_work", bufs=3, space="PSUM") as ps_work,
            tc.tile_pool(name="ps_av", bufs=4, space="PSUM") as ps_av,
            tc.tile_pool(name="ps_sums", bufs=1, space="PSUM") as ps_sums,
        ):
            # ---------- resident small tensors ----------
            wuqT_sb = pres.tile([128, HPC * NLQ], f32r, tag="wuqT")
            wuk_sb = pres.tile([128, HPC * NLKV], f32r, tag="wuk")
            wqr2T_sb = pres.tile([128, HPC * 4 * 2 * DHR], f32r, tag="wqr2T")
            cos2_sb = pres.tile([DHR, T], f32r, tag="cos2")
            sin2_sb = pres.tile([DHR, T], f32r, tag="sin2")
            cos2o_sb = pres.tile([DHR, TOK], f32r, tag="cos2o")
            sin2o_sb = pres.tile([DHR, TOK], f32r, tag="sin2o")
            k_eff_sb = pres.tile([128, HPC * 4 * NLKV], f32r, tag="k_eff")
            v_eff_sb = pres.tile([128, 4 * HPC * HS], f32r, tag="v_eff")
            ones_sb = pres.tile([128, 1], f32, tag="ones")
            ones_r = pres.tile([128, 1], f32r, tag="ones_r")
            onesc_sb = pres.tile([1, 128], f32, tag="onesc")
            onesc_r = pres.tile([1, 128], f32r, tag="onesc_r")
            mask_r = pres.tile([128, 128], f32r, tag="mask_r")

            for hh in range(HPC):
                nc.sync.dma_start(wuqT_sb[:, hh * NLQ:(hh + 1) * NLQ],
                                  wuqT[hh * HS:(hh + 1) * HS, :])
                nc.sync.dma_start(wuk_sb[:, hh * NLKV:(hh + 1) * NLKV],
                                  wuk[hh * HS:(hh + 1) * HS, :])
                for qc in range(4):
                    nc.sync.dma_start(
                        wqr2T_sb[:, (hh * 4 + qc) * 128:(hh * 4 + qc + 1) * 128],
                        wqr2T[hh * NLQ + qc * 128: hh * NLQ + (qc + 1) * 128, :])
            nc.sync.dma_start(cos2_sb[:], cos2[:])
            nc.sync.dma_start(sin2_sb[:], sin2[:])
            nc.sync.dma_start(cos2o_sb[:], cos2o[:])
            nc.sync.dma_start(sin2o_sb[:], sin2o[:])

            nc.gpsimd.memset(ones_sb[:], 1.0)
            nc.vector.tensor_copy(ones_r[:], ones_sb[:])
            nc.gpsimd.memset(onesc_sb[:], 1.0)
            nc.vector.tensor_copy(onesc_r[:], onesc_sb[:])
            # causal mask for diagonal 128-col block: keep where i <= y
            nc.sync.dma_start(mask_r[:], maskp[:])

            # DRAM bounce buffers for the AllGather
            agin = dram.tile([SH_ROWS, TOK], f32r)
            agout = dram.tile([NCORES * SH_ROWS, TOK], f32r)

            # ---------- phase 0a: local preprocessing ----------
            with tc.tile_pool(name="p0", bufs=1) as p0:
                xT_sb = p0.tile([128, CC * TOK], f32r, tag="xT")
                wdqT_sb = p0.tile([128, CC * NLQ], f32r, tag="wdqT")
                wdkvT_sb = p0.tile([128, CC * NLKV], f32r, tag="wdkvT")
                wkr2T_sb = p0.tile([128, CC * 2 * DHR], f32r, tag="wkr2T")
                cqT_own = p0.tile([128, 4 * TOK], f32r, tag="cqT_own")
                ckvT_own = p0.tile([128, 4 * TOK], f32r, tag="ckvT_own")
                ckv_own = p0.tile([128, 4 * NLKV], f32r, tag="ckv_own")
                krT_own = p0.tile([DHR, TOK], f32r, tag="krT_own")
                rtmp = p0.tile([DHR, 2 * TOK], f32, tag="rtmp")

                for cc in range(CC):
                    nc.sync.dma_start(xT_sb[:, cc * TOK:(cc + 1) * TOK],
                                      xT[cc * 128:(cc + 1) * 128, :])
                    nc.sync.dma_start(wdqT_sb[:, cc * NLQ:(cc + 1) * NLQ],
                                      wdqT[cc * 128:(cc + 1) * 128, :])
                    nc.sync.dma_start(wdkvT_sb[:, cc * NLKV:(cc + 1) * NLKV],
                                      wdkvT[cc * 128:(cc + 1) * 128, :])
                    nc.sync.dma_start(wkr2T_sb[:, cc * 2 * DHR:(cc + 1) * 2 * DHR],
                                      wkr2T[cc * 128:(cc + 1) * 128, :])

                # c_qT_own[q, t] and c_kvT_own[k, t]
                for qt in range(4):
                    pq = ps_work.tile([128, TOK], f32, tag="work")
                    for cc in range(CC):
                        nc.tensor.matmul(
                            pq[:],
                            wdqT_sb[:, cc * NLQ + qt * 128: cc * NLQ + (qt + 1) * 128],
                            xT_sb[:, cc * TOK:(cc + 1) * TOK],
                            start=(cc == 0), stop=(cc == CC - 1))
                    nc.vector.tensor_copy(cqT_own[:, qt * TOK:(qt + 1) * TOK], pq[:])
                for kt in range(4):
                    pk = ps_work.tile([128, TOK], f32, tag="work")
                    for cc in range(CC):
                        nc.tensor.matmul(
                            pk[:],
                            wdkvT_sb[:, cc * NLKV + kt * 128: cc * NLKV + (kt + 1) * 128],
                            xT_sb[:, cc * TOK:(cc + 1) * TOK],
                            start=(cc == 0), stop=(cc == CC - 1))
                    nc.vector.tensor_copy(ckvT_own[:, kt * TOK:(kt + 1) * TOK], pk[:])
                # c_kv_own[t, k] directly (t-major) for the AV matmul lhsT
                for tt in range(4):
                    pk = ps_work.tile([128, NLKV], f32, tag="work")
                    for cc in range(CC):
                        nc.tensor.matmul(
                            pk[:],
                            xT_sb[:, cc * TOK + tt * 128: cc * TOK + (tt + 1) * 128],
                            wdkvT_sb[:, cc * NLKV:(cc + 1) * NLKV],
                            start=(cc == 0), stop=(cc == CC - 1))
                    nc.vector.tensor_copy(ckv_own[:, tt * NLKV:(tt + 1) * NLKV], pk[:])
                # k_r (roped): rows 0..63 plain, 64..127 pair-swapped copy
                pr = ps_work.tile([128, TOK], f32, tag="work")
                for cc in range(CC):
                    nc.tensor.matmul(
                        pr[:],
                        wkr2T_sb[:, cc * 2 * DHR:(cc + 1) * 2 * DHR],
                        xT_sb[:, cc * TOK:(cc + 1) * TOK],
                        start=(cc == 0), stop=(cc == CC - 1))
                nc.vector.tensor_tensor(rtmp[:, :TOK], pr[:DHR, :], cos2o_sb[:], mult)
                nc.vector.tensor_tensor(rtmp[:, TOK:], pr[DHR:, :], sin2o_sb[:], mult)
                nc.vector.tensor_tensor(krT_own[:], rtmp[:, :TOK], rtmp[:, TOK:], add)

                # pack into the AG input bounce
                for qt in range(4):
                    nc.gpsimd.dma_start(
                        agin[ROW_CQ + qt * 128: ROW_CQ + (qt + 1) * 128, :],
                        cqT_own[:, qt * TOK:(qt + 1) * TOK])
                    nc.gpsimd.dma_start(
                        agin[ROW_CKVT + qt * 128: ROW_CKVT + (qt + 1) * 128, :],
                        ckvT_own[:, qt * TOK:(qt + 1) * TOK])
                    nc.gpsimd.dma_start(
                        agin[ROW_CKV + qt * 128: ROW_CKV + (qt + 1) * 128, :],
                        ckv_own[:, qt * NLKV:(qt + 1) * NLKV])
                nc.gpsimd.dma_start(agin[ROW_KR: ROW_KR + DHR, :], krT_own[:])

            nc.gpsimd.collective_compute(
                "AllGather", mybir.AluOpType.bypass,
                replica_groups=[list(range(NCORES))],
                ins=[agin.opt()], outs=[agout.opt()])

            # ---------- phase 0b: absorbed weight prep ----------
            # k_eff[h] = W_uq_h @ W_uk_h  -> [q, k'] slabs (q-tile major)
            for hh in range(HPC):
                for qt in range(4):
                    pk = ps_work.tile([128, NLKV], f32, tag="work")
                    nc.tensor.matmul(
                        pk[:],
                        wuqT_sb[:, hh * NLQ + qt * 128: hh * NLQ + (qt + 1) * 128],
                        wuk_sb[:, hh * NLKV:(hh + 1) * NLKV],
                        start=True, stop=True)
                    nc.vector.tensor_copy(
                        k_eff_sb[:, (hh * 4 + qt) * NLKV:(hh * 4 + qt + 1) * NLKV],
                        pk[:])
            with tc.tile_pool(name="pw", bufs=1) as pw:
                wuv_sb = pw.tile([128, CC * NLKV], f32r, tag="wuv")
                wo2T_sb = pw.tile([128, CC * HPC * HS], f32r, tag="wo2T")
                for cc in range(CC):
                    nc.sync.dma_start(wuv_sb[:, cc * NLKV:(cc + 1) * NLKV],
                                      wuv[cc * 128:(cc + 1) * 128, :])
                    nc.sync.dma_start(
                        wo2T_sb[:, cc * HPC * HS:(cc + 1) * HPC * HS],
                        wo2T[cc * 128:(cc + 1) * 128, :])
                # v_eff[k', d2] (d2 = both heads' HS)
                for kt in range(4):
                    pv = ps_work.tile([128, HPC * HS], f32, tag="work")
                    for cc in range(CC):
                        nc.tensor.matmul(
                            pv[:],
                            wuv_sb[:, cc * NLKV + kt * 128: cc * NLKV + (kt + 1) * 128],
                            wo2T_sb[:, cc * HPC * HS:(cc + 1) * HPC * HS],
                            start=(cc == 0), stop=(cc == CC - 1))
                    nc.vector.tensor_copy(
                        v_eff_sb[:, kt * HPC * HS:(kt + 1) * HPC * HS], pv[:])

            # ---------- phase 2: attention per (batch, head) ----------
            with (
                tc.tile_pool(name="pb", bufs=1) as pb,
                tc.tile_pool(name="ph", bufs=2) as ph,
                tc.tile_pool(name="pex", bufs=6) as pex,
                tc.tile_pool(name="py", bufs=2) as py,
            ):
                for b in range(B):
                    cqT_b = pb.tile([128, 4 * T], f32r, tag="cqT_b")
                    ckvT_b = pb.tile([128, 4 * T], f32r, tag="ckvT_b")
                    ckv_b = pb.tile([128, 8 * NLKV], f32r, tag="ckv_b")
                    krT_b = pb.tile([DHR, T], f32r, tag="krT_b")
                    for half in range(2):
                        r = 2 * b + half
                        base = r * SH_ROWS
                        for qt in range(4):
                            nc.sync.dma_start(
                                cqT_b[:, qt * T + half * TOK: qt * T + (half + 1) * TOK],
                                agout[base + ROW_CQ + qt * 128: base + ROW_CQ + (qt + 1) * 128, :])
                            nc.sync.dma_start(
                                ckvT_b[:, qt * T + half * TOK: qt * T + (half + 1) * TOK],
                                agout[base + ROW_CKVT + qt * 128: base + ROW_CKVT + (qt + 1) * 128, :])
                            sc = half * 4 + qt  # s-chunk index 0..7
                            nc.sync.dma_start(
                                ckv_b[:, sc * NLKV:(sc + 1) * NLKV],
                                agout[base + ROW_CKV + qt * 128: base + ROW_CKV + (qt + 1) * 128, :])
                        nc.sync.dma_start(
                            krT_b[:, half * TOK:(half + 1) * TOK],
                            agout[base + ROW_KR: base + ROW_KR + DHR, :])

                    for hh in range(HPC):
                        # q_effT[k', t] for the whole batch
                        qeT_sb = ph.tile([128, 4 * T], f32r, tag="qeT")
                        for ts2 in range(2):
                            for kt in range(4):
                                pq = ps_work.tile([128, 512], f32, tag="work")
                                for qc in range(4):
                                    nc.tensor.matmul(
                                        pq[:],
                                        k_eff_sb[:, (hh * 4 + qc) * NLKV + kt * 128:
                                                 (hh * 4 + qc) * NLKV + (kt + 1) * 128],
                                        cqT_b[:, qc * T + ts2 * 512: qc * T + (ts2 + 1) * 512],
                                        start=(qc == 0), stop=(qc == 3))
                                nc.vector.tensor_copy(
                                    qeT_sb[:, kt * T + ts2 * 512: kt * T + (ts2 + 1) * 512],
                                    pq[:])
                        # roped q_rT[d, t]
                        qrT_sb = ph.tile([DHR, T], f32r, tag="qrT")
                        qrtmp = ph.tile([DHR, 2 * 512], f32, tag="qrtmp")
                        for ts2 in range(2):
                            pq = ps_work.tile([128, 512], f32, tag="work")
                            for qc in range(4):
                                nc.tensor.matmul(
                                    pq[:],
                                    wqr2T_sb[:, (hh * 4 + qc) * 128:(hh * 4 + qc + 1) * 128],
                                    cqT_b[:, qc * T + ts2 * 512: qc * T + (ts2 + 1) * 512],
                                    start=(qc == 0), stop=(qc == 3))
                            sl = slice(ts2 * 512, (ts2 + 1) * 512)
                            nc.vector.tensor_tensor(
                                qrtmp[:, :512], pq[:DHR, :], cos2_sb[:, sl], mult)
                            nc.vector.tensor_tensor(
                                qrtmp[:, 512:], pq[DHR:, :], sin2_sb[:, sl], mult)
                            nc.vector.tensor_tensor(
                                qrT_sb[:, sl], qrtmp[:, :512], qrtmp[:, 512:], add)

                        for ts2 in range(2):
                            t0 = ts2 * 512
                            n_s = 4 * ts2 + 4
                            av_ps = [ps_av.tile([128, 512], f32, tag="av",
                                                name=f"av_{b}_{hh}_{ts2}_{kt}")
                                     for kt in range(4)]
                            sums_ps = ps_sums.tile([1, 512], f32, tag="sums")
                            for j in range(n_s):
                                t_off = max(0, 128 * j - t0)
                                njt = 512 - t_off
                                tsl = slice(t0 + t_off, t0 + 512)
                                lg = ps_work.tile([128, 512], f32, tag="work")
                                for kc in range(4):
                                    nc.tensor.matmul(
                                        lg[:, :njt],
                                        ckvT_b[:, kc * T + j * 128: kc * T + (j + 1) * 128],
                                        qeT_sb[:, kc * T + t0 + t_off: kc * T + t0 + 512],
                                        start=(kc == 0), stop=False)
                                nc.tensor.matmul(
                                    lg[:, :njt],
                                    krT_b[:, j * 128:(j + 1) * 128],
                                    qrT_sb[:, tsl],
                                    start=False, stop=True)
                                ex = pex.tile([128, 512], f32r, tag="ex")
                                nc.scalar.activation(ex[:, :njt], lg[:, :njt],
                                                     Exp, scale=SCALE)
                                if 128 * j >= t0:
                                    nc.vector.tensor_tensor(
                                        ex[:, :128], ex[:, :128], mask_r[:], mult)
                                last = (j == n_s - 1)
                                for kt in range(4):
                                    nc.tensor.matmul(
                                        av_ps[kt][:, t_off:],
                                        ckv_b[:, j * NLKV + kt * 128: j * NLKV + (kt + 1) * 128],
                                        ex[:, :njt],
                                        start=(j == 0), stop=last)
                                nc.tensor.matmul(
                                    sums_ps[:, t_off:], ones_r[:], ex[:, :njt],
                                    start=(j == 0), stop=last)
                            # normalize and project out
                            recip32 = py.tile([1, 512], f32, tag="recip32")
                            nc.vector.reciprocal(recip32[:], sums_ps[:])
                            recip = py.tile([1, 512], f32r, tag="recip")
                            nc.vector.tensor_copy(recip[:], recip32[:])
                            bc_ps = ps_work.tile([128, 512], f32, tag="work")
                            nc.tensor.matmul(bc_ps[:], onesc_r[:], recip[:],
                                             start=True, stop=True)
                            bc_sb = py.tile([128, 512], f32, tag="bc")
                            nc.vector.tensor_copy(bc_sb[:], bc_ps[:])
                            av_sb = ph.tile([128, 4 * 512], f32r, tag="av_sb")
                            for kt in range(4):
                                nc.vector.tensor_copy(
                                    av_sb[:, kt * 512:(kt + 1) * 512], av_ps[kt][:])
                            yp = ps_work.tile([128, 512], f32, tag="work")
                            for kt in range(4):
                                nc.tensor.matmul(
                                    yp[:],
                                    v_eff_sb[:, kt * HPC * HS + hh * HS:
                                             kt * HPC * HS + (hh + 1) * HS],
                                    av_sb[:, kt * 512:(kt + 1) * 512],
                                    start=(kt == 0), stop=(kt == 3))
                            y_sb = py.tile([128, 512], f32, tag="y")
                            nc.vector.tensor_tensor(y_sb[:], yp[:], bc_sb[:], mult)
                            nc.sync.dma_start(
                                out[hh * HS:(hh + 1) * HS,
                                    b * T + t0: b * T + t0 + 512],
                                y_sb[:])
    nc.compile()
    return nc


def _pairswap(w):
    idx = np.arange(w.shape[0]).reshape(-1, 2)[:, ::-1].reshape(-1)
    return w[idx]


def _make_in_maps(x, W_dq, W_uq, W_dkv, W_uk, W_uv, W_o, W_qr, W_kr,
                  freqs_cos, freqs_sin):
    f4 = np.float32
    wdqT = np.ascontiguousarray(W_dq.T, dtype=f4)
    wdkvT = np.ascontiguousarray(W_dkv.T, dtype=f4)
    wkr2T = np.ascontiguousarray(
        np.concatenate([W_kr.T, _pairswap(W_kr).T], axis=1), dtype=f4)
    wuv = np.ascontiguousarray(W_uv, dtype=f4)
    uq = W_uq.reshape(NLQ, NH, HS)
    uk = W_uk.reshape(NH, HS, NLKV)
    cos2 = np.repeat(freqs_cos.T, 2, axis=0).astype(f4)          # [DHR, T]
    sin_half = freqs_sin.T.astype(f4)                            # [DHR/2, T]
    sin2 = np.empty((DHR, T), dtype=f4)
    sin2[0::2] = -sin_half
    sin2[1::2] = sin_half

    in_maps = []
    for i in range(NCORES):
        b_own, half = divmod(i, 2)
        t0 = half * TOK
        heads = [HPC * i + hh for hh in range(HPC)]
        wuqT = np.concatenate([uq[:, h, :].T for h in heads], axis=0)
        wukh = np.concatenate([uk[h] for h in heads], axis=0)
        wqr2T = np.concatenate([
            np.concatenate(
                [W_qr[h * DHR:(h + 1) * DHR].T,
                 _pairswap(W_qr[h * DHR:(h + 1) * DHR]).T], axis=1)
            for h in heads], axis=0)
        wo2T = np.ascontiguousarray(
            W_o[heads[0] * HS: (heads[-1] + 1) * HS, :].T, dtype=f4)
        in_maps.append({
            "xT": np.ascontiguousarray(x[b_own, t0:t0 + TOK, :].T, dtype=f4),
            "wdqT": wdqT, "wdkvT": wdkvT, "wkr2T": wkr2T,
            "wuqT": wuqT.astype(f4), "wuk": wukh.astype(f4),
            "wqr2T": wqr2T.astype(f4), "wo2T": wo2T, "wuv": wuv,
            "cos2": cos2, "sin2": sin2,
            "cos2o": np.ascontiguousarray(cos2[:, t0:t0 + TOK]),
            "sin2o": np.ascontiguousarray(sin2[:, t0:t0 + TOK]),
            "maskp": np.tril(np.ones((128, 128), dtype=f4)).T.copy(),
        })
    return in_maps


def _assemble(results):
    y = np.empty((B, T, C), dtype=np.float32)
    for i in range(NCORES):
        o = results[i]["out"]  # [HPC*HS, B*T]
        for hh in range(HPC):
            h = HPC * i + hh
            blk = o[hh * HS:(hh + 1) * HS, :].reshape(HS, B, T)
            y[:, :, h * HS:(h + 1) * HS] = blk.transpose(1, 2, 0)
    return y


def kernel(**inputs):
    from concourse import bass_utils
    if "nc" not in _cache:
        _cache["nc"] = _build()
    nc = _cache["nc"]
    in_maps = _make_in_maps(**{k: np.asarray(v) for k, v in inputs.items()})
    res = bass_utils.run_bass_kernel_spmd(nc, in_maps, core_ids=list(range(NCORES)))
    return _assemble(res.results)


# revision 51
# speedup vs baseline: 685.4999x; 685.4999x over previous
"""MLA attention (DeepSeek-style, absorbed weights) on 8 TRN2 NeuronCores.

Sharding: heads are tensor-parallel (2 heads/core; in the absorbed
formulation output channel blocks are per-head independent, so no output
all-reduce). The shared latent projections (c_q, c_kv, roped k_r) are
token-parallel (512 tokens/core) and exchanged with two AllGathers (c_q
first, so q_eff compute overlaps the second gather).

Precision: preprocessing and the output projection run in float32r
(TF32-like, 1 PE cycle/row); attention-side activations (c_q/c_kv/k_r
latents, q_eff, exp weights) are bf16 — same PE speed, half the DMA/SBUF/
collective bytes. PSUM accumulation is always fp32.

Attention runs in a transposed orientation logitsT[s, t]: both QK^T and AV
matmuls need no transposes; softmax needs no max subtraction (logits are
O(1) here) and column sums come from an appended ones-row matmul.
"""

import math

import numpy as np

B, T, C = 4, 1024, 2048
NH, HS = 16, 128
NLQ = NLKV = 512
DHR = 64
NCORES = 8
HPC = NH // NCORES          # heads per core
TOK = (B * T) // NCORES     # tokens per core (half a batch)
SCALE = 1.0 / math.sqrt(HS + DHR)

# AllGather shard layouts. Every packed tensor's shard is exactly its SBUF
# tile layout [128, X] row-major, so pack and load DMAs are contiguous
# (DMA dispatch cost scales with descriptor count = discontiguous runs).
AG1_W = 4 * TOK                 # c_qT_own   [128, (q, t)]
COL_CKVT = 0                    # c_kvT_own  [128, (kc, s)]
COL_CKV = 4 * TOK               # c_kv_own   [128, (tt, k)]
COL_KR = 8 * TOK                # k_rT_own   [64, t] in a 512-col block
AG2_W = 8 * TOK + TOK           # 4608

_cache = {}


def _build(loop_k=None, sim_single=False):
    """Build the SPMD kernel. loop_k: if set, wrap phase 2 (attention) in a
    For_i(0, loop_k) hardware loop — used only for timing amplification.
    sim_single: single-core no-collective variant (gathers fed as inputs)
    for TimelineSim cost-model analysis."""
    import contextlib

    import concourse.mybir as mybir
    import concourse.tile as tile
    from concourse import bacc

    f32 = mybir.dt.float32
    f32r = mybir.dt.float32r
    bf16 = mybir.dt.bfloat16
    Exp = mybir.ActivationFunctionType.Exp
    Copy = mybir.ActivationFunctionType.Copy
    mult = mybir.AluOpType.mult
    add = mybir.AluOpType.add

    nc = bacc.Bacc(trn_type="TRN2", num_devices=1 if sim_single else NCORES)
    P = nc.declare_dram_parameter

    CC = C // 128  # 16 c-chunks

    # all weight/input params arrive pre-arranged in SBUF slab layout
    # [128, n_chunks * W] (host does the transform — contiguous DMAs here)
    xT = P("xT", [128, CC * TOK], f32r, isOutput=False)
    wdqT = P("wdqT", [128, CC * NLQ], f32r, isOutput=False)
    wdkvT = P("wdkvT", [128, CC * NLKV], f32r, isOutput=False)
    wkr2T = P("wkr2T", [128, CC * 2 * DHR], f32r, isOutput=False)
    wuqT = P("wuqT", [128, HPC * NLQ], f32r, isOutput=False)
    wuk = P("wuk", [128, HPC * NLKV], f32r, isOutput=False)
    wqr2T = P("wqr2T", [128, HPC * 4 * 2 * DHR], bf16, isOutput=False)
    wo2T = P("wo2T", [128, CC * HPC * HS], f32r, isOutput=False)
    wuv = P("wuv", [128, CC * NLKV], f32r, isOutput=False)
    cos2 = P("cos2", [DHR, T], f32, isOutput=False)
    sin2 = P("sin2", [DHR, T], f32, isOutput=False)
    cos2o = P("cos2o", [DHR, TOK], f32, isOutput=False)
    sin2o = P("sin2o", [DHR, TOK], f32, isOutput=False)
    maskp = P("maskp", [128, 128], bf16, isOutput=False)
    out = P("out", [HPC * HS, B * T], f32, isOutput=True)
    ag1_p = ag2_p = None
    if sim_single:
        ag1_p = P("ag1_p", [NCORES * 128, AG1_W], bf16, isOutput=False)
        ag2_p = P("ag2_p", [NCORES * 128, AG2_W], bf16, isOutput=False)

    with tile.TileContext(nc) as tc:
        with (
            tc.tile_pool(name="pres", bufs=1) as pres,
            tc.tile_pool(name="dram", bufs=1, space="DRAM") as dram,
            tc.tile_pool(name="ps_work", bufs=3, space="PSUM") as ps_work,
            tc.tile_pool(name="ps_av", bufs=4, space="PSUM") as ps_av,
            tc.tile_pool(name="ps_sums", bufs=1, space="PSUM") as ps_sums,
        ):
            # ---------- resident small tensors ----------
            wuqT_sb = pres.tile([128, HPC * NLQ], f32r, tag="wuqT")
            wuk_sb = pres.tile([128, HPC * NLKV], f32r, tag="wuk")
            wqr2T_sb = pres.tile([128, HPC * 4 * 2 * DHR], bf16, tag="wqr2T")
            cos2_sb = pres.tile([DHR, T], f32, tag="cos2")
            sin2_sb = pres.tile([DHR, T], f32, tag="sin2")
            cos2o_sb = pres.tile([DHR, TOK], f32, tag="cos2o")
            sin2o_sb = pres.tile([DHR, TOK], f32, tag="sin2o")
            k_eff_sb = pres.tile([128, HPC * 4 * NLKV], bf16, tag="k_eff")
            v_eff_sb = pres.tile([128, 4 * HPC * HS], f32r, tag="v_eff")
            ones_sb = pres.tile([128, 1], f32, tag="ones")
            ones_r = pres.tile([128, 1], bf16, tag="ones_r")
            onesc_sb = pres.tile([1, 128], f32, tag="onesc")
            onesc_r = pres.tile([1, 128], f32r, tag="onesc_r")
            mask_r = pres.tile([128, 128], bf16, tag="mask_r")

            nc.sync.dma_start(wuqT_sb[:], wuqT[:, :])
            nc.sync.dma_start(wuk_sb[:], wuk[:, :])
            nc.sync.dma_start(wqr2T_sb[:], wqr2T[:, :])
            nc.sync.dma_start(cos2_sb[:], cos2[:])
            nc.sync.dma_start(sin2_sb[:], sin2[:])
            nc.sync.dma_start(cos2o_sb[:], cos2o[:])
            nc.sync.dma_start(sin2o_sb[:], sin2o[:])

            nc.gpsimd.memset(ones_sb[:], 1.0)
            nc.vector.tensor_copy(ones_r[:], ones_sb[:])
            nc.gpsimd.memset(onesc_sb[:], 1.0)
            nc.vector.tensor_copy(onesc_r[:], onesc_sb[:])
            # causal mask for a diagonal 128-col block: keep where s_i <= t_y
            nc.sync.dma_start(mask_r[:], maskp[:])

            # DRAM bounce buffers for the AllGathers
            agin1 = dram.tile([128, AG1_W], bf16)
            agin2 = dram.tile([128, AG2_W], bf16)
            agout1 = ag1_p if sim_single else dram.tile(
                [NCORES * 128, AG1_W], bf16, addr_space="Shared")
            agout2 = ag2_p if sim_single else dram.tile(
                [NCORES * 128, AG2_W], bf16, addr_space="Shared")

            # ---------- phase 0a: local preprocessing ----------
            with tc.tile_pool(name="p0", bufs=1) as p0:
                xT_sb = p0.tile([128, CC * TOK], f32r, tag="xT")
                wdqT_sb = p0.tile([128, CC * NLQ], f32r, tag="wdqT")
                wdkvT_sb = p0.tile([128, CC * NLKV], f32r, tag="wdkvT")
                wkr2T_sb = p0.tile([128, CC * 2 * DHR], f32r, tag="wkr2T")
                cqT_own = p0.tile([128, 4 * TOK], bf16, tag="cqT_own")
                ckvT_own = p0.tile([128, 4 * TOK], bf16, tag="ckvT_own")
                ckv_own = p0.tile([128, 4 * NLKV], bf16, tag="ckv_own")
                krT_own = p0.tile([DHR, TOK], bf16, tag="krT_own")
                rtmp = p0.tile([DHR, 2 * TOK], f32, tag="rtmp")

                for qr_ in range(4):
                    csl = slice(qr_ * 4 * TOK, (qr_ + 1) * 4 * TOK)
                    nc.sync.dma_start(xT_sb[:, csl], xT[:, csl])
                    wsl = slice(qr_ * 4 * NLQ, (qr_ + 1) * 4 * NLQ)
                    nc.sync.dma_start(wdqT_sb[:, wsl], wdqT[:, wsl])
                    nc.sync.dma_start(wdkvT_sb[:, wsl], wdkvT[:, wsl])
                    ksl = slice(qr_ * 8 * DHR, (qr_ + 1) * 8 * DHR)
                    nc.sync.dma_start(wkr2T_sb[:, ksl], wkr2T[:, ksl])

                # c_qT_own[q, t] then its AllGather right away
                for qt in range(4):
                    pq = ps_work.tile([128, TOK], f32, tag="work")
                    for cc in range(CC):
                        nc.tensor.matmul(
                            pq[:],
                            wdqT_sb[:, cc * NLQ + qt * 128: cc * NLQ + (qt + 1) * 128],
                            xT_sb[:, cc * TOK:(cc + 1) * TOK],
                            start=(cc == 0), stop=(cc == CC - 1))
                    nc.vector.tensor_copy(cqT_own[:, qt * TOK:(qt + 1) * TOK], pq[:])
                nc.gpsimd.dma_start(agin1[:, :], cqT_own[:])
                if not sim_single:
                    nc.gpsimd.collective_compute(
                        "AllGather", mybir.AluOpType.bypass,
                        replica_groups=[list(range(NCORES))],
                        ins=[agin1.opt()], outs=[agout1.opt()])

                for kt in range(4):
                    pk = ps_work.tile([128, TOK], f32, tag="work")
                    for cc in range(CC):
                        nc.tensor.matmul(
                            pk[:],
                            wdkvT_sb[:, cc * NLKV + kt * 128: cc * NLKV + (kt + 1) * 128],
                            xT_sb[:, cc * TOK:(cc + 1) * TOK],
                            start=(cc == 0), stop=(cc == CC - 1))
                    nc.vector.tensor_copy(ckvT_own[:, kt * TOK:(kt + 1) * TOK], pk[:])
                # c_kv_own[t, k] directly (t-major) for the AV matmul lhsT
                for tt in range(4):
                    pk = ps_work.tile([128, NLKV], f32, tag="work")
                    for cc in range(CC):
                        nc.tensor.matmul(
                            pk[:],
                            xT_sb[:, cc * TOK + tt * 128: cc * TOK + (tt + 1) * 128],
                            wdkvT_sb[:, cc * NLKV:(cc + 1) * NLKV],
                            start=(cc == 0), stop=(cc == CC - 1))
                    nc.vector.tensor_copy(ckv_own[:, tt * NLKV:(tt + 1) * NLKV], pk[:])
                nc.gpsimd.dma_start(agin2[:, COL_CKVT:COL_CKVT + 4 * TOK],
                                    ckvT_own[:])
                nc.gpsimd.dma_start(agin2[:, COL_CKV:COL_CKV + 4 * TOK],
                                    ckv_own[:])
                # k_r (roped): rows 0..63 plain, 64..127 pair-swapped copy
                pr = ps_work.tile([128, TOK], f32, tag="work")
                for cc in range(CC):
                    nc.tensor.matmul(
                        pr[:],
                        wkr2T_sb[:, cc * 2 * DHR:(cc + 1) * 2 * DHR],
                        xT_sb[:, cc * TOK:(cc + 1) * TOK],
                        start=(cc == 0), stop=(cc == CC - 1))
                nc.vector.tensor_tensor(rtmp[:, :TOK], pr[:DHR, :], cos2o_sb[:], mult)
                nc.vector.tensor_tensor(rtmp[:, TOK:], pr[DHR:, :], sin2o_sb[:], mult)
                nc.vector.tensor_tensor(krT_own[:], rtmp[:, :TOK], rtmp[:, TOK:], add)
                nc.gpsimd.dma_start(agin2[:DHR, COL_KR:COL_KR + TOK], krT_own[:])

            if not sim_single:
                nc.gpsimd.collective_compute(
                    "AllGather", mybir.AluOpType.bypass,
                    replica_groups=[list(range(NCORES))],
                    ins=[agin2.opt()], outs=[agout2.opt()])

            # ---------- phase 0b: absorbed weight prep ----------
            # k_eff[h] = W_uq_h @ W_uk_h  -> [q, k'] slabs (q-tile major)
            for hh in range(HPC):
                for qt in range(4):
                    pk = ps_work.tile([128, NLKV], f32, tag="work")
                    nc.tensor.matmul(
                        pk[:],
                        wuqT_sb[:, hh * NLQ + qt * 128: hh * NLQ + (qt + 1) * 128],
                        wuk_sb[:, hh * NLKV:(hh + 1) * NLKV],
                        start=True, stop=True)
                    nc.scalar.activation(
                        k_eff_sb[:, (hh * 4 + qt) * NLKV:(hh * 4 + qt + 1) * NLKV],
                        pk[:], Copy)
            with tc.tile_pool(name="pw", bufs=1) as pw:
                wuv_sb = pw.tile([128, CC * NLKV], f32r, tag="wuv")
                wo2T_sb = pw.tile([128, CC * HPC * HS], f32r, tag="wo2T")
                nc.sync.dma_start(wuv_sb[:], wuv[:, :])
                nc.sync.dma_start(wo2T_sb[:], wo2T[:, :])
                # v_eff[k', d2] (d2 = both heads' HS)
                for kt in range(4):
                    pv = ps_work.tile([128, HPC * HS], f32, tag="work")
                    for cc in range(CC):
                        nc.tensor.matmul(
                            pv[:],
                            wuv_sb[:, cc * NLKV + kt * 128: cc * NLKV + (kt + 1) * 128],
                            wo2T_sb[:, cc * HPC * HS:(cc + 1) * HPC * HS],
                            start=(cc == 0), stop=(cc == CC - 1))
                    nc.scalar.activation(
                        v_eff_sb[:, kt * HPC * HS:(kt + 1) * HPC * HS], pv[:], Copy)

            # ---------- phase 2: attention per (batch, head) ----------
            with (
                tc.tile_pool(name="pb", bufs=2) as pb,
                tc.tile_pool(name="ph", bufs=2) as ph,
                tc.tile_pool(name="pex", bufs=12) as pex,
                tc.tile_pool(name="py", bufs=2) as py,
                tc.For_i(0, loop_k, 1) if loop_k else contextlib.nullcontext(),
            ):
                for b in range(B):
                    cqT_b = pb.tile([128, 4 * T], bf16, tag="cqT_b")
                    ckvT_b = pb.tile([128, 4 * T], bf16, tag="ckvT_b")
                    ckv_b = pb.tile([128, 8 * NLKV], bf16, tag="ckv_b")
                    krT_b = pb.tile([DHR, T], bf16, tag="krT_b")
                    # fully contiguous per-rank loads: the shard layout in the
                    # bounce equals the SBUF layout
                    ag1r = agout1.ap() if sim_single else agout1[:]
                    ag2r = agout2.ap() if sim_single else agout2[:]
                    for half in range(2):
                        r = 2 * b + half
                        nc.sync.dma_start(
                            cqT_b[:, half * 4 * TOK:(half + 1) * 4 * TOK],
                            ag1r[r * 128:(r + 1) * 128, :])
                        nc.sync.dma_start(
                            ckvT_b[:, half * 4 * TOK:(half + 1) * 4 * TOK],
                            ag2r[r * 128:(r + 1) * 128, COL_CKVT:COL_CKVT + 4 * TOK])
                        nc.sync.dma_start(
                            ckv_b[:, half * 4 * TOK:(half + 1) * 4 * TOK],
                            ag2r[r * 128:(r + 1) * 128, COL_CKV:COL_CKV + 4 * TOK])
                        nc.sync.dma_start(
                            krT_b[:, half * TOK:(half + 1) * TOK],
                            ag2r[r * 128: r * 128 + DHR, COL_KR:COL_KR + TOK])

                    for hh in range(HPC):
                        # q_effT[k', t] for the whole batch
                        qeT_sb = ph.tile([128, 4 * T], bf16, tag="qeT")
                        for ts2 in range(2):
                            for kt in range(4):
                                pq = ps_work.tile([128, 512], f32, tag="work")
                                for qc in range(4):
                                    nc.tensor.matmul(
                                        pq[:],
                                        k_eff_sb[:, (hh * 4 + qc) * NLKV + kt * 128:
                                                 (hh * 4 + qc) * NLKV + (kt + 1) * 128],
                                        cqT_b[:, ts2 * 2048 + qc * 512:
                                              ts2 * 2048 + (qc + 1) * 512],
                                        start=(qc == 0), stop=(qc == 3))
                                nc.scalar.activation(
                                    qeT_sb[:, kt * T + ts2 * 512: kt * T + (ts2 + 1) * 512],
                                    pq[:], Copy)
                        # roped q_rT[d, t]
                        qrT_sb = ph.tile([DHR, T], bf16, tag="qrT")
                        qrtmp = ph.tile([DHR, 2 * 512], f32, tag="qrtmp")
                        for ts2 in range(2):
                            pq = ps_work.tile([128, 512], f32, tag="work")
                            for qc in range(4):
                                nc.tensor.matmul(
                                    pq[:],
                                    wqr2T_sb[:, (hh * 4 + qc) * 128:(hh * 4 + qc + 1) * 128],
                                    cqT_b[:, ts2 * 2048 + qc * 512:
                                          ts2 * 2048 + (qc + 1) * 512],
                                    start=(qc == 0), stop=(qc == 3))
                            sl = slice(ts2 * 512, (ts2 + 1) * 512)
                            nc.vector.tensor_tensor(
                                qrtmp[:, :512], pq[:DHR, :], cos2_sb[:, sl], mult)
                            nc.vector.tensor_tensor(
                                qrtmp[:, 512:], pq[DHR:, :], sin2_sb[:, sl], mult)
                            nc.vector.tensor_tensor(
                                qrT_sb[:, sl], qrtmp[:, :512], qrtmp[:, 512:], add)

                        y2_sb = py.tile([128, 2 * 512], f32, tag="y2")
                        for ts2 in range(2):
                            t0 = ts2 * 512
                            n_s = 4 * ts2 + 4
                            av_ps = [ps_av.tile([128, 512], f32, tag="av",
                                                name=f"av_{b}_{hh}_{ts2}_{kt}")
                                     for kt in range(4)]
                            sums_ps = ps_sums.tile([1, 512], f32, tag="sums")
                            # pass 1: all logits + exp for this t-span. Keeping
                            # the AV matmuls out of the logits->exp dependency
                            # chain lets the PE run ahead instead of stalling
                            # on ACT each chunk (stalls also keep the HAM
                            # clock-gate cold).
                            exs = []
                            for j in range(n_s):
                                t_off = max(0, 128 * j - t0)
                                njt = 512 - t_off
                                tsl = slice(t0 + t_off, t0 + 512)
                                lg = ps_work.tile([128, 512], f32, tag="work")
                                sb2 = (j // 4) * 2048 + (j % 4) * 128
                                for kc in range(4):
                                    nc.tensor.matmul(
                                        lg[:, :njt],
                                        ckvT_b[:, sb2 + kc * 512: sb2 + kc * 512 + 128],
                                        qeT_sb[:, kc * T + t0 + t_off: kc * T + t0 + 512],
                                        start=(kc == 0), stop=False)
                                nc.tensor.matmul(
                                    lg[:, :njt],
                                    krT_b[:, j * 128:(j + 1) * 128],
                                    qrT_sb[:, tsl],
                                    start=False, stop=True)
                                ex = pex.tile([128, 512], bf16, tag="ex",
                                              name=f"ex_{b}_{hh}_{ts2}_{j}")
                                nc.scalar.activation(ex[:, :njt], lg[:, :njt],
                                                     Exp, scale=SCALE)
                                if 128 * j >= t0:
                                    nc.gpsimd.tensor_tensor(
                                        ex[:, :128], ex[:, :128], mask_r[:], mult)
                                exs.append((ex, t_off, njt))
                            # pass 2: AV + sums accumulation over all chunks
                            for j, (ex, t_off, njt) in enumerate(exs):
                                last = (j == n_s - 1)
                                vb2 = (j // 4) * 2048 + (j % 4) * 512
                                for kt in range(4):
                                    nc.tensor.matmul(
                                        av_ps[kt][:, t_off:],
                                        ckv_b[:, vb2 + kt * 128: vb2 + (kt + 1) * 128],
                                        ex[:, :njt],
                                        start=(j == 0), stop=last)
                                nc.tensor.matmul(
                                    sums_ps[:, t_off:], ones_r[:], ex[:, :njt],
                                    start=(j == 0), stop=last)
                            # normalize and project out
                            recip = py.tile([1, 512], f32r, tag="recip")
                            with nc.allow_low_precision(
                                    reason="f32r normalization scale is plenty"):
                                nc.vector.reciprocal(recip[:], sums_ps[:])
                            bc_ps = ps_work.tile([128, 512], f32, tag="work")
                            nc.tensor.matmul(bc_ps[:], onesc_r[:], recip[:],
                                             start=True, stop=True)
                            bc_sb = py.tile([128, 512], f32, tag="bc")
                            nc.scalar.activation(bc_sb[:], bc_ps[:], Copy)
                            av_sb = ph.tile([128, 4 * 512], f32r, tag="av_sb")
                            for kt in range(4):
                                nc.scalar.activation(
                                    av_sb[:, kt * 512:(kt + 1) * 512], av_ps[kt][:],
                                    Copy)
                            yp = ps_work.tile([128, 512], f32, tag="work")
                            for kt in range(4):
                                nc.tensor.matmul(
                                    yp[:],
                                    v_eff_sb[:, kt * HPC * HS + hh * HS:
                                             kt * HPC * HS + (hh + 1) * HS],
                                    av_sb[:, kt * 512:(kt + 1) * 512],
                                    start=(kt == 0), stop=(kt == 3))
                            nc.vector.tensor_tensor(
                                y2_sb[:, t0:t0 + 512], yp[:], bc_sb[:], mult)
                        nc.sync.dma_start(
                            out[hh * HS:(hh + 1) * HS, b * T: (b + 1) * T],
                            y2_sb[:])
    nc.compile()
    return nc


def _pairswap(w):
    idx = np.arange(w.shape[0]).reshape(-1, 2)[:, ::-1].reshape(-1)
    return w[idx]


def _slab(m, dtype=np.float32):
    """[n*128, W] row-major -> SBUF slab layout [128, n*W]."""
    n = m.shape[0] // 128
    return np.ascontiguousarray(
        m.reshape(n, 128, m.shape[1]).transpose(1, 0, 2).reshape(128, -1),
        dtype=dtype)


def _make_in_maps(x, W_dq, W_uq, W_dkv, W_uk, W_uv, W_o, W_qr, W_kr,
                  freqs_cos, freqs_sin):
    import ml_dtypes
    f4 = np.float32
    bf = ml_dtypes.bfloat16
    wdqT = _slab(W_dq.T)
    wdkvT = _slab(W_dkv.T)
    wkr2T = _slab(np.concatenate([W_kr.T, _pairswap(W_kr).T], axis=1))
    wuv = _slab(W_uv)
    uq = W_uq.reshape(NLQ, NH, HS)
    uk = W_uk.reshape(NH, HS, NLKV)
    cos2 = np.repeat(freqs_cos.T, 2, axis=0).astype(f4)          # [DHR, T]
    sin_half = freqs_sin.T.astype(f4)                            # [DHR/2, T]
    sin2 = np.empty((DHR, T), dtype=f4)
    sin2[0::2] = -sin_half
    sin2[1::2] = sin_half

    in_maps = []
    for i in range(NCORES):
        b_own, half = divmod(i, 2)
        t0 = half * TOK
        heads = [HPC * i + hh for hh in range(HPC)]
        wuqT = _slab(np.concatenate([uq[:, h, :].T for h in heads], axis=0))
        wukh = _slab(np.concatenate([uk[h] for h in heads], axis=0))
        wqr2T = _slab(np.concatenate([
            np.concatenate(
                [W_qr[h * DHR:(h + 1) * DHR].T,
                 _pairswap(W_qr[h * DHR:(h + 1) * DHR]).T], axis=1)
            for h in heads], axis=0), dtype=bf)
        wo2T = _slab(W_o[heads[0] * HS: (heads[-1] + 1) * HS, :].T)
        in_maps.append({
            "xT": _slab(x[b_own, t0:t0 + TOK, :].T),
            "wdqT": wdqT, "wdkvT": wdkvT, "wkr2T": wkr2T,
            "wuqT": wuqT, "wuk": wukh,
            "wqr2T": wqr2T, "wo2T": wo2T, "wuv": wuv,
            "cos2": cos2, "sin2": sin2,
            "cos2o": np.ascontiguousarray(cos2[:, t0:t0 + TOK]),
            "sin2o": np.ascontiguousarray(sin2[:, t0:t0 + TOK]),
            "maskp": np.triu(np.ones((128, 128))).astype(bf),
        })
    return in_maps


def _assemble(results):
    y = np.empty((B, T, C), dtype=np.float32)
    for i in range(NCORES):
        o = results[i]["out"]  # [HPC*HS, B*T]
        for hh in range(HPC):
            h = HPC * i + hh
            blk = o[hh * HS:(hh + 1) * HS, :].reshape(HS, B, T)
            y[:, :, h * HS:(h + 1) * HS] = blk.transpose(1, 2, 0)
    return y


def kernel(**inputs):
    from concourse import bass_utils
    if "nc" not in _cache:
        _cache["nc"] = _build()
    nc = _cache["nc"]
    in_maps = _make_in_maps(**{k: np.asarray(v) for k, v in inputs.items()})
    res = bass_utils.run_bass_kernel_spmd(nc, in_maps, core_ids=list(range(NCORES)))
    return _assemble(res.results)
